# revision 33
# baseline (speedup 1.0000x reference)
"""GQA attention layer (B=2,S=2048,D=2048,H=16,KV=4,HD=128) on 8 trn2 cores.

Sharding: core = (b, g) for b in {0,1} (batch), g in {0..3} (kv group).
Each core computes q-heads 4g..4g+3 + kv head g for batch b, producing a
partial o-projection [S, D] (bf16); the host sums the 4 partials per batch.

Per-core kernel: everything in transposed layout (head_dim on partitions),
bf16 matmuls with fp32 accumulation, softmax without max-subtraction
(logits bounded after RMSNorm), causal block skipping. Partition-dim
reductions (rms-norm sum-of-squares, softmax denominator) via ones-matmul
with M=128 so the result is already broadcast across partitions;
reciprocals/rsqrt computed on ScalarE as exp(-a*ln(x)).

v is projected in transposed layout (like k) so its weight loads hide
behind N=512 matmuls, then moved to natural [s, hd] layout with XBAR
DMA transposes (natural-layout v projection is load-dominated: N=128
matmuls cannot hide the 128-row weight loads). Input DMAs are split
across both HWDGE queues (sync: wk -> x0 quarters -> wq head-pair
halves -> x1,x3; scalar: scales -> wv -> rope tables -> tri -> x2 ->
wo), ordered so the tensor engine's first matmuls depend on minimal
bytes; all four x chunks have resident buffers so transfers never wait
on compute. cos/sin tables and the output partial are bf16 (error
budget allows it; halves that DMA traffic). O-projection phases are
emitted last = lowest priority, so the priority-heap scheduler uses
their dependency-free matmuls to fill exp-latency bubbles in the
attention tail; they share the projection PSUM pool (free by then)
while attention accumulation keeps its own, and output copies stay on
DVE because ACT saturates with exp during the B3/O overlap.
"""
import numpy as np
import ml_dtypes

B, S, DM = 2, 2048, 2048
H, KV, HD = 16, 4, 128
G = H // KV
THETA = 10000.0
EPS = 1e-6

P = 128         # partitions
CH = 512        # s-chunk (matmul N)
NCH = S // CH   # 4
KT = DM // P    # 16 contraction tiles
NST = S // P    # 16 s-tiles

_CACHE = {}
# extra kwargs for run_bass_kernel_spmd (test harness sets trace/tmpdir here)
_RUN_KWARGS = {}


def _build_nc():
    from concourse import bacc, mybir
    import concourse.tile as tile
    from contextlib import ExitStack

    f32 = mybir.dt.float32
    bf16 = mybir.dt.bfloat16
    Act = mybir.ActivationFunctionType

    nc = bacc.Bacc()
    d_xt = nc.declare_dram_parameter("xt", [NCH, P, KT, CH], bf16, isOutput=False)
    # wq stored head-pair-major: two contiguous 1MB blocks of [P, KT, 2, HD]
    # so q0/q1 projections can complete after the first block lands
    d_wq = nc.declare_dram_parameter("wq4", [2, P, KT, 2, HD], bf16, isOutput=False)
    d_wk = nc.declare_dram_parameter("wk1", [P, KT, HD], bf16, isOutput=False)
    d_wv = nc.declare_dram_parameter("wv1", [P, KT, HD], bf16, isOutput=False)
    d_wo = nc.declare_dram_parameter("wo4", [HD, G, DM], bf16, isOutput=False)
    d_qs = nc.declare_dram_parameter("qsc", [HD, 1], f32, isOutput=False)
    d_ks = nc.declare_dram_parameter("ksc", [HD, 1], f32, isOutput=False)
    d_psw = nc.declare_dram_parameter("psw", [P, P], bf16, isOutput=False)
    d_cos = nc.declare_dram_parameter("cos_t", [P, S], bf16, isOutput=False)
    d_sin = nc.declare_dram_parameter("sin_t", [P, S], bf16, isOutput=False)
    d_tri = nc.declare_dram_parameter("tri", [P, P], bf16, isOutput=False)
    d_out = nc.declare_dram_parameter("o_part", [S, DM], bf16, isOutput=True)

    with tile.TileContext(nc) as tc, ExitStack() as ctx:
        const = ctx.enter_context(tc.tile_pool(name="const", bufs=1))
        xin = ctx.enter_context(tc.tile_pool(name="xin", bufs=4))
        work = ctx.enter_context(tc.tile_pool(name="work", bufs=4))
        vtp = ctx.enter_context(tc.tile_pool(name="vtp", bufs=2))
        osbp = ctx.enter_context(tc.tile_pool(name="osbp", bufs=4))
        pbp = ctx.enter_context(tc.tile_pool(name="pbp", bufs=8))
        wnorm = ctx.enter_context(tc.tile_pool(name="wnorm", bufs=3))
        # PSUM: 8 banks. pa_ops shared by projections (A) + o-proj (O) so
        # attention accumulation (p_att) never blocks o-proj bubble-filling.
        p_pa = ctx.enter_context(tc.tile_pool(name="p_pa", bufs=2, space="PSUM"))
        p_sc = ctx.enter_context(tc.tile_pool(name="p_sc", bufs=3, space="PSUM"))
        p_red = ctx.enter_context(tc.tile_pool(name="p_red", bufs=1, space="PSUM"))
        p_att = ctx.enter_context(tc.tile_pool(name="p_att", bufs=2, space="PSUM"))

        # ---- persistent SBUF + input DMA schedule ----
        # sync ring (Q1): wk first (unblocks the first k-proj ~3us before the
        # scalar ring boots), then x0 quarters, wq halves, x1, x3.
        # scalar ring (Q10): wv + small tables, then x2 (rebalances the load
        # so Q10 doesn't idle while Q1 still streams x), then wo.
        wk_sb = const.tile([P, KT, HD], bf16, tag="wk_sb")
        nc.sync.dma_start(out=wk_sb, in_=d_wk[:])
        xt0 = xin.tile([P, KT, CH], bf16, tag="xt_c")
        for i in range(4):
            nc.sync.dma_start(out=xt0[:, 4 * i:4 * i + 4], in_=d_xt[0, :, 4 * i:4 * i + 4])
        wq_sb = [const.tile([P, KT, 2, HD], bf16, tag=f"wq_sb{i}", name=f"wq_sb{i}")
                 for i in range(2)]
        nc.sync.dma_start(out=wq_sb[0], in_=d_wq[0])
        nc.sync.dma_start(out=wq_sb[1], in_=d_wq[1])
        # all four x chunks are resident (xin bufs=4), so every x DMA streams
        # immediately instead of waiting for an earlier chunk to be consumed
        x_later = []
        for c in (1, 2, 3):
            xt_c = xin.tile([P, KT, CH], bf16, tag="xt_c", name=f"xt_c{c}")
            x_later.append(xt_c)
        nc.sync.dma_start(out=x_later[0], in_=d_xt[1])
        nc.sync.dma_start(out=x_later[2], in_=d_xt[3])

        qsc_sb = const.tile([HD, 1], f32, tag="qsc_sb")
        nc.scalar.dma_start(out=qsc_sb, in_=d_qs[:])
        ksc_sb = const.tile([HD, 1], f32, tag="ksc_sb")
        nc.scalar.dma_start(out=ksc_sb, in_=d_ks[:])
        wv_sb = const.tile([P, KT, HD], bf16, tag="wv_sb")
        nc.scalar.dma_start(out=wv_sb, in_=d_wv[:])
        psw_sb = const.tile([P, P], bf16, tag="psw_sb")
        nc.scalar.dma_start(out=psw_sb, in_=d_psw[:])
        cos_sb = const.tile([P, S], bf16, tag="cos_sb")
        nc.scalar.dma_start(out=cos_sb, in_=d_cos[:])
        sin_sb = const.tile([P, S], bf16, tag="sin_sb")
        nc.scalar.dma_start(out=sin_sb, in_=d_sin[:])
        tri_sb = const.tile([P, P], bf16, tag="tri_sb")
        nc.scalar.dma_start(out=tri_sb, in_=d_tri[:])
        nc.scalar.dma_start(out=x_later[1], in_=d_xt[2])
        wo_sb = const.tile([P, G, DM], bf16, tag="wo_sb")
        nc.scalar.dma_start(out=wo_sb, in_=d_wo[:])

        ones_bb = const.tile([P, P], bf16, tag="ones_bb")
        nc.vector.memset(ones_bb, 1.0)
        eps_q = const.tile([P, 1], f32, tag="eps_q")
        nc.vector.memset(eps_q, float(HD * EPS))
        eps_k = const.tile([P, 1], f32, tag="eps_k")
        nc.vector.memset(eps_k, float(EPS))

        # roped q heads / k / v / normalized att, persistent
        qro = [const.tile([P, S], bf16, tag=f"qro{h}", name=f"qro{h}") for h in range(G)]
        kro = const.tile([P, S], bf16, tag="kro")
        v_sb = const.tile([P, NST, HD], bf16, tag="v_sb")
        att_sb = [const.tile([P, S], bf16, tag=f"att{h}", name=f"att{h}") for h in range(G)]

        # ---- Phase A (projections+rmsnorm+rope) per chunk ----
        def emit_A(c):
                cs = slice(c * CH, (c + 1) * CH)
                xt_c = xt0 if c == 0 else x_later[c - 1]
                # order: k first (smallest weights, arrives earliest), then v
                # (transposed; weight loads hidden), then q heads.
                for h in (G, G + 1, 0, 1, 2, 3):
                    is_q = h < G
                    is_v = h == G + 1
                    ps_q = p_pa.tile([P, CH], f32, tag="pa")
                    for kt in range(KT):
                        if is_q:
                            lhs = wq_sb[h // 2][:, kt, h % 2, :]
                        elif is_v:
                            lhs = wv_sb[:, kt, :]
                        else:
                            lhs = wk_sb[:, kt, :]
                        nc.tensor.matmul(
                            ps_q, lhsT=lhs, rhs=xt_c[:, kt],
                            start=(kt == 0), stop=(kt == KT - 1),
                        )
                    if is_v:
                        # vT chunk -> bf16 SBUF -> natural [s, hd] via XBAR
                        vt_sb = vtp.tile([P, CH], bf16, tag="vt_sb")
                        nc.vector.tensor_copy(vt_sb, ps_q)
                        for st in range(4):
                            nc.sync.dma_start_transpose(
                                out=v_sb[:, 4 * c + st, :],
                                in_=vt_sb[:, st * P:(st + 1) * P])
                        continue
                    # rmsnorm: sumsq over hd via ones-matmul (M=128 -> broadcast rows)
                    qsq = wnorm.tile([P, CH], bf16, tag="qsq")
                    nc.scalar.activation(out=qsq, in_=ps_q, func=Act.Square)
                    ss = p_sc.tile([P, CH], f32, tag="sc")
                    nc.tensor.matmul(ss, lhsT=ones_bb, rhs=qsq, start=True, stop=True)
                    ln = wnorm.tile([P, CH], f32, tag="ln")
                    if is_q:
                        # rn = 1/sqrt(sumsq + HD*eps) == rmsnorm_scale * HD^-0.5
                        nc.scalar.activation(out=ln, in_=ss, func=Act.Ln,
                                             scale=1.0, bias=eps_q)
                    else:
                        nc.scalar.activation(out=ln, in_=ss, func=Act.Ln,
                                             scale=1.0 / HD, bias=eps_k)
                    rn = wnorm.tile([P, CH], f32, tag="rn")
                    nc.scalar.activation(out=rn, in_=ln, func=Act.Exp, scale=-0.5)
                    qs = work.tile([P, CH], bf16, tag="qs")
                    nc.vector.scalar_tensor_tensor(
                        out=qs, in0=ps_q, scalar=(qsc_sb if is_q else ksc_sb), in1=rn,
                        op0=mybir.AluOpType.mult, op1=mybir.AluOpType.mult)
                    # rope: out = qs*cos + swap(qs)*sin_signed (swap via PE permute)
                    rot = p_sc.tile([P, CH], f32, tag="sc")
                    nc.tensor.matmul(rot, lhsT=psw_sb, rhs=qs, start=True, stop=True)
                    t1 = work.tile([P, CH], f32, tag="t1")
                    nc.vector.tensor_mul(t1, qs, cos_sb[:, cs])
                    u = work.tile([P, CH], f32, tag="u")
                    nc.vector.tensor_mul(u, rot, sin_sb[:, cs])
                    dst = qro[h] if is_q else kro
                    nc.vector.tensor_add(dst[:, cs], t1, u)

        # ---- Phase B (attention) per chunk ----
        def emit_B(c):
                for h in range(G):
                    cs = slice(c * CH, (c + 1) * CH)
                    attps = p_att.tile([P, CH], f32, tag="att")
                    csum = p_red.tile([P, CH], f32, tag="cs")
                    tmax = 4 * c + 4
                    for t in range(tmax):
                        j = t - 4 * c
                        off = P * j if j > 0 else 0
                        sc = p_sc.tile([P, CH], f32, tag="sc")
                        nc.tensor.matmul(
                            sc[:, off:], lhsT=kro[:, t * P:(t + 1) * P],
                            rhs=qro[h][:, c * CH + off:(c + 1) * CH],
                            start=True, stop=True,
                        )
                        pb = pbp.tile([P, CH], bf16, tag="pb")
                        nc.scalar.activation(out=pb[:, off:], in_=sc[:, off:], func=Act.Exp)
                        if j >= 0:
                            # diagonal block: zero where sq < sk in the 128-col group
                            nc.vector.tensor_mul(pb[:, off:off + P], pb[:, off:off + P], tri_sb)
                        nc.tensor.matmul(csum[:, off:], lhsT=ones_bb, rhs=pb[:, off:],
                                         start=(t == 0), stop=(t == tmax - 1),
                                         skip_group_check=True)
                        nc.tensor.matmul(attps[:, off:], lhsT=v_sb[:, t, :], rhs=pb[:, off:],
                                         start=(t == 0), stop=(t == tmax - 1),
                                         skip_group_check=True)
                    # normalize: att = attps / colsum (reciprocal on DVE, ~2ulp)
                    rcp = wnorm.tile([P, CH], f32, tag="rn")
                    scr = wnorm.tile([P, CH], f32, tag="ln")
                    nc.vector.reciprocal_approx_accurate(out=rcp, in_=csum, scratch=scr)
                    nc.vector.tensor_mul(att_sb[h][:, cs], attps, rcp)

        # ---- Phase O (output projection) per chunk ----
        def emit_O(c):
                for st in range(4 * c, 4 * c + 4):
                    for mc in range(NCH):
                        ops = p_pa.tile([P, CH], f32, tag="pa")
                        for h in range(G):
                            nc.tensor.matmul(
                                ops, lhsT=att_sb[h][:, st * P:(st + 1) * P],
                                rhs=wo_sb[:, h, mc * CH:(mc + 1) * CH],
                                start=(h == 0), stop=(h == G - 1),
                            )
                        osb = osbp.tile([P, CH], bf16, tag="osb")
                        nc.vector.tensor_copy(osb, ops)
                        nc.sync.dma_start(
                            out=d_out[st * P:(st + 1) * P, mc * CH:(mc + 1) * CH], in_=osb)

        # A/B interleaved so B's ACT-heavy stretches overlap A's PE-heavy
        # matmuls; O phases emitted last = lowest priority, so the scheduler
        # uses their (dependency-free) matmuls to fill exp-latency bubbles.
        emit_A(0)
        emit_B(0)
        emit_A(1)
        emit_B(1)
        emit_A(2)
        emit_B(2)
        emit_A(3)
        emit_B(3)
        emit_O(0)
        emit_O(1)
        emit_O(2)
        emit_O(3)

    # Pin every activation to the one table set that contains all functions
    # we use (exp/ln/copy/square), so the ACT engine never swaps tables.
    # Indices must stay aligned with act_info.json, so other sets are kept
    # in place but emptied (the pass then can't pick them).
    from concourse import bacc as bacc_mod
    orig_tables = bacc_mod.get_activation_tables
    target = "natural_log_exp_and_others"

    def unified_tables(arch):
        t = orig_tables(arch)
        assert target in t
        return {k: (v if k == target else set()) for k, v in t.items()}

    bacc_mod.get_activation_tables = unified_tables
    try:
        nc.compile()
    finally:
        bacc_mod.get_activation_tables = orig_tables
    return nc


def _get_nc():
    if "nc" not in _CACHE:
        _CACHE["nc"] = _build_nc()
    return _CACHE["nc"]


def _rope_tables():
    inv_ts = THETA ** (-np.arange(HD // 2, dtype=np.float64) / (HD // 2))
    ang = np.arange(S, dtype=np.float64)[None, :] * inv_ts[:, None]  # [64, S]
    cos64 = np.cos(ang)
    sin64 = np.sin(ang)
    cos_t = np.concatenate([cos64, cos64], 0).astype(np.float32)
    # rotate-then-multiply signs: top rows get -sin, bottom +sin
    sin_t = np.concatenate([-sin64, sin64], 0).astype(np.float32)
    return cos_t, sin_t


def kernel(x, wq, wk, wv, wo, q_scale, k_scale):
    bf = ml_dtypes.bfloat16
    x = np.asarray(x, np.float32)
    wq = np.asarray(wq, np.float32)
    wk = np.asarray(wk, np.float32)
    wv = np.asarray(wv, np.float32)
    wo = np.asarray(wo, np.float32)
    q_scale = np.asarray(q_scale, np.float32)
    k_scale = np.asarray(k_scale, np.float32)

    from concourse.bass_utils import run_bass_kernel_spmd

    nc = _get_nc()
    cos_t, sin_t = _rope_tables()
    half = P // 2
    psw = np.zeros((P, P), np.float32)
    psw[np.arange(half) + half, np.arange(half)] = 1.0
    psw[np.arange(half), np.arange(half) + half] = 1.0
    tri = (np.arange(P)[None, :] >= np.arange(P)[:, None]).astype(np.float32)

    in_maps = []
    for core in range(8):
        b, g = divmod(core, 4)
        in_maps.append({
            "xt": np.ascontiguousarray(
                x[b].T.reshape(KT, P, NCH, CH).transpose(2, 1, 0, 3)).astype(bf),
            "wq4": np.ascontiguousarray(
                wq[:, 4 * g:4 * g + 4, :].reshape(KT, P, 2, 2, HD)
                .transpose(2, 1, 0, 3, 4)).astype(bf),
            "wk1": np.ascontiguousarray(
                wk[:, g, :].reshape(KT, P, HD).transpose(1, 0, 2)).astype(bf),
            "wv1": np.ascontiguousarray(
                wv[:, g, :].reshape(KT, P, HD).transpose(1, 0, 2)).astype(bf),
            "wo4": np.ascontiguousarray(np.transpose(wo[4 * g:4 * g + 4], (1, 0, 2))).astype(bf),
            "qsc": q_scale.reshape(HD, 1),
            "ksc": k_scale.reshape(HD, 1),
            "psw": psw.astype(bf),
            "cos_t": cos_t.astype(bf),
            "sin_t": sin_t.astype(bf),
            "tri": tri.astype(bf),
        })

    res = run_bass_kernel_spmd(nc, in_maps, list(range(8)), **_RUN_KWARGS)
    _CACHE["last_res"] = res
    out = np.zeros((B, S, DM), np.float32)
    for core in range(8):
        out[core // 4] += res.results[core]["o_part"].astype(np.float32)
    return out


# revision 35
# speedup vs baseline: 1.0139x; 1.0139x over previous
"""GQA attention layer (B=2,S=2048,D=2048,H=16,KV=4,HD=128) on 8 trn2 cores.

Sharding: core = (b, g) for b in {0,1} (batch), g in {0..3} (kv group).
Each core computes q-heads 4g..4g+3 + kv head g for batch b, producing a
partial o-projection [S, D] (bf16); the host sums the 4 partials per batch.

Per-core kernel: everything in transposed layout (head_dim on partitions),
bf16 matmuls with fp32 accumulation, softmax without max-subtraction
(logits bounded after RMSNorm), causal block skipping. Partition-dim
reductions (rms-norm sum-of-squares, softmax denominator) via ones-matmul
with M=128 so the result is already broadcast across partitions;
reciprocals/rsqrt computed on ScalarE as exp(-a*ln(x)).

v is projected in transposed layout (like k) so its weight loads hide
behind N=512 matmuls, then moved to natural [s, hd] layout with XBAR
DMA transposes (natural-layout v projection is load-dominated: N=128
matmuls cannot hide the 128-row weight loads). Input DMAs are split
across both HWDGE queues (sync: wk -> x0 quarters -> wq head-pair
halves -> x1,x3; scalar: scales -> wv -> rope tables -> tri -> x2 ->
wo), ordered so the tensor engine's first matmuls depend on minimal
bytes; all four x chunks have resident buffers so transfers never wait
on compute. cos/sin tables and the output partial are bf16 (error
budget allows it; halves that DMA traffic). O-projection phases are
emitted last = lowest priority, so the priority-heap scheduler uses
their dependency-free matmuls to fill exp-latency bubbles in the
attention tail; they share the projection PSUM pool (free by then)
while attention accumulation keeps its own, and output copies stay on
DVE because ACT saturates with exp during the B3/O overlap.
"""
import numpy as np
import ml_dtypes

B, S, DM = 2, 2048, 2048
H, KV, HD = 16, 4, 128
G = H // KV
THETA = 10000.0
EPS = 1e-6

P = 128         # partitions
CH = 512        # s-chunk (matmul N)
NCH = S // CH   # 4
KT = DM // P    # 16 contraction tiles
NST = S // P    # 16 s-tiles

_CACHE = {}
# extra kwargs for run_bass_kernel_spmd (test harness sets trace/tmpdir here)
_RUN_KWARGS = {}


def _build_nc():
    from concourse import bacc, mybir
    import concourse.tile as tile
    from contextlib import ExitStack

    f32 = mybir.dt.float32
    bf16 = mybir.dt.bfloat16
    Act = mybir.ActivationFunctionType

    nc = bacc.Bacc()
    d_xt = nc.declare_dram_parameter("xt", [NCH, P, KT, CH], bf16, isOutput=False)
    # wq stored head-pair-major: two contiguous 1MB blocks of [P, KT, 2, HD]
    # so q0/q1 projections can complete after the first block lands
    d_wq = nc.declare_dram_parameter("wq4", [2, P, KT, 2, HD], bf16, isOutput=False)
    d_wk = nc.declare_dram_parameter("wk1", [P, KT, HD], bf16, isOutput=False)
    d_wv = nc.declare_dram_parameter("wv1", [P, KT, HD], bf16, isOutput=False)
    d_wo = nc.declare_dram_parameter("wo4", [HD, G, DM], bf16, isOutput=False)
    d_qs = nc.declare_dram_parameter("qsc", [HD, 1], f32, isOutput=False)
    d_ks = nc.declare_dram_parameter("ksc", [HD, 1], f32, isOutput=False)
    d_psw = nc.declare_dram_parameter("psw", [P, P], bf16, isOutput=False)
    d_cos = nc.declare_dram_parameter("cos_t", [P, S], bf16, isOutput=False)
    d_sin = nc.declare_dram_parameter("sin_t", [P, S], bf16, isOutput=False)
    d_tri = nc.declare_dram_parameter("tri", [P, P], bf16, isOutput=False)
    d_out = nc.declare_dram_parameter("o_part", [S, DM], bf16, isOutput=True)

    with tile.TileContext(nc) as tc, ExitStack() as ctx:
        const = ctx.enter_context(tc.tile_pool(name="const", bufs=1))
        xin = ctx.enter_context(tc.tile_pool(name="xin", bufs=4))
        work = ctx.enter_context(tc.tile_pool(name="work", bufs=4))
        vtp = ctx.enter_context(tc.tile_pool(name="vtp", bufs=2))
        osbp = ctx.enter_context(tc.tile_pool(name="osbp", bufs=4))
        pbp = ctx.enter_context(tc.tile_pool(name="pbp", bufs=8))
        wnorm = ctx.enter_context(tc.tile_pool(name="wnorm", bufs=3))
        # PSUM: 8 banks. pa_ops shared by projections (A) + o-proj (O) so
        # attention accumulation (p_att) never blocks o-proj bubble-filling.
        p_pa = ctx.enter_context(tc.tile_pool(name="p_pa", bufs=2, space="PSUM"))
        p_sc = ctx.enter_context(tc.tile_pool(name="p_sc", bufs=3, space="PSUM"))
        p_red = ctx.enter_context(tc.tile_pool(name="p_red", bufs=1, space="PSUM"))
        p_att = ctx.enter_context(tc.tile_pool(name="p_att", bufs=2, space="PSUM"))

        # ---- persistent SBUF + input DMA schedule ----
        # sync ring (Q1): wk first (unblocks the first k-proj ~3us before the
        # scalar ring boots), then x0 quarters, wq halves, x1, x3.
        # scalar ring (Q10): wv + small tables, then x2 (rebalances the load
        # so Q10 doesn't idle while Q1 still streams x), then wo.
        wk_sb = const.tile([P, KT, HD], bf16, tag="wk_sb")
        nc.sync.dma_start(out=wk_sb, in_=d_wk[:])
        xt0 = xin.tile([P, KT, CH], bf16, tag="xt_c")
        for i in range(4):
            nc.sync.dma_start(out=xt0[:, 4 * i:4 * i + 4], in_=d_xt[0, :, 4 * i:4 * i + 4])
        wq_sb = [const.tile([P, KT, 2, HD], bf16, tag=f"wq_sb{i}", name=f"wq_sb{i}")
                 for i in range(2)]
        nc.sync.dma_start(out=wq_sb[0], in_=d_wq[0])
        nc.sync.dma_start(out=wq_sb[1], in_=d_wq[1])
        # all four x chunks are resident (xin bufs=4), so every x DMA streams
        # immediately instead of waiting for an earlier chunk to be consumed
        x_later = []
        for c in (1, 2, 3):
            xt_c = xin.tile([P, KT, CH], bf16, tag="xt_c", name=f"xt_c{c}")
            x_later.append(xt_c)
        nc.sync.dma_start(out=x_later[0], in_=d_xt[1])
        nc.sync.dma_start(out=x_later[2], in_=d_xt[3])

        qsc_sb = const.tile([HD, 1], f32, tag="qsc_sb")
        nc.scalar.dma_start(out=qsc_sb, in_=d_qs[:])
        ksc_sb = const.tile([HD, 1], f32, tag="ksc_sb")
        nc.scalar.dma_start(out=ksc_sb, in_=d_ks[:])
        wv_sb = const.tile([P, KT, HD], bf16, tag="wv_sb")
        nc.scalar.dma_start(out=wv_sb, in_=d_wv[:])
        psw_sb = const.tile([P, P], bf16, tag="psw_sb")
        nc.scalar.dma_start(out=psw_sb, in_=d_psw[:])
        cos_sb = const.tile([P, S], bf16, tag="cos_sb")
        nc.scalar.dma_start(out=cos_sb, in_=d_cos[:])
        sin_sb = const.tile([P, S], bf16, tag="sin_sb")
        nc.scalar.dma_start(out=sin_sb, in_=d_sin[:])
        tri_sb = const.tile([P, P], bf16, tag="tri_sb")
        nc.scalar.dma_start(out=tri_sb, in_=d_tri[:])
        nc.scalar.dma_start(out=x_later[1], in_=d_xt[2])
        wo_sb = const.tile([P, G, DM], bf16, tag="wo_sb")
        nc.scalar.dma_start(out=wo_sb, in_=d_wo[:])

        ones_bb = const.tile([P, P], bf16, tag="ones_bb")
        nc.vector.memset(ones_bb, 1.0)
        eps_q = const.tile([P, 1], f32, tag="eps_q")
        nc.vector.memset(eps_q, float(HD * EPS))
        eps_k = const.tile([P, 1], f32, tag="eps_k")
        nc.vector.memset(eps_k, float(EPS))

        # roped q heads / k / v / normalized att, persistent
        qro = [const.tile([P, S], bf16, tag=f"qro{h}", name=f"qro{h}") for h in range(G)]
        kro = const.tile([P, S], bf16, tag="kro")
        v_sb = const.tile([P, NST, HD], bf16, tag="v_sb")
        att_sb = [const.tile([P, S], bf16, tag=f"att{h}", name=f"att{h}") for h in range(G)]

        # ---- Phase A (projections+rmsnorm+rope) per chunk ----
        def emit_A(c):
                cs = slice(c * CH, (c + 1) * CH)
                xt_c = xt0 if c == 0 else x_later[c - 1]
                # order: k first (smallest weights, arrives earliest), then v
                # (transposed; weight loads hidden), then q heads.
                for h in (G, G + 1, 0, 1, 2, 3):
                    is_q = h < G
                    is_v = h == G + 1
                    ps_q = p_pa.tile([P, CH], f32, tag="pa")
                    for kt in range(KT):
                        if is_q:
                            lhs = wq_sb[h // 2][:, kt, h % 2, :]
                        elif is_v:
                            lhs = wv_sb[:, kt, :]
                        else:
                            lhs = wk_sb[:, kt, :]
                        nc.tensor.matmul(
                            ps_q, lhsT=lhs, rhs=xt_c[:, kt],
                            start=(kt == 0), stop=(kt == KT - 1),
                        )
                    if is_v:
                        # vT chunk -> bf16 SBUF -> natural [s, hd] via XBAR
                        vt_sb = vtp.tile([P, CH], bf16, tag="vt_sb")
                        nc.vector.tensor_copy(vt_sb, ps_q)
                        for st in range(4):
                            nc.sync.dma_start_transpose(
                                out=v_sb[:, 4 * c + st, :],
                                in_=vt_sb[:, st * P:(st + 1) * P])
                        continue
                    # rmsnorm: sumsq over hd via ones-matmul (M=128 -> broadcast rows)
                    qsq = wnorm.tile([P, CH], bf16, tag="qsq")
                    nc.scalar.activation(out=qsq, in_=ps_q, func=Act.Square)
                    ss = p_sc.tile([P, CH], f32, tag="sc")
                    nc.tensor.matmul(ss, lhsT=ones_bb, rhs=qsq, start=True, stop=True)
                    ln = wnorm.tile([P, CH], f32, tag="ln")
                    if is_q:
                        # rn = 1/sqrt(sumsq + HD*eps) == rmsnorm_scale * HD^-0.5
                        nc.scalar.activation(out=ln, in_=ss, func=Act.Ln,
                                             scale=1.0, bias=eps_q)
                    else:
                        nc.scalar.activation(out=ln, in_=ss, func=Act.Ln,
                                             scale=1.0 / HD, bias=eps_k)
                    rn = wnorm.tile([P, CH], f32, tag="rn")
                    nc.scalar.activation(out=rn, in_=ln, func=Act.Exp, scale=-0.5)
                    qs = work.tile([P, CH], bf16, tag="qs")
                    nc.vector.scalar_tensor_tensor(
                        out=qs, in0=ps_q, scalar=(qsc_sb if is_q else ksc_sb), in1=rn,
                        op0=mybir.AluOpType.mult, op1=mybir.AluOpType.mult)
                    # rope: out = qs*cos + swap(qs)*sin_signed (swap via PE permute)
                    rot = p_sc.tile([P, CH], f32, tag="sc")
                    nc.tensor.matmul(rot, lhsT=psw_sb, rhs=qs, start=True, stop=True)
                    t1 = work.tile([P, CH], f32, tag="t1")
                    nc.vector.tensor_mul(t1, qs, cos_sb[:, cs])
                    u = work.tile([P, CH], f32, tag="u")
                    nc.vector.tensor_mul(u, rot, sin_sb[:, cs])
                    dst = qro[h] if is_q else kro
                    nc.vector.tensor_add(dst[:, cs], t1, u)

        # ---- Phase B (attention) per chunk ----
        def emit_B(c):
                for h in range(G):
                    cs = slice(c * CH, (c + 1) * CH)
                    attps = p_att.tile([P, CH], f32, tag="att")
                    csum = p_red.tile([P, CH], f32, tag="cs")
                    tmax = 4 * c + 4
                    for t in range(tmax):
                        j = t - 4 * c
                        off = P * j if j > 0 else 0
                        sc = p_sc.tile([P, CH], f32, tag="sc")
                        nc.tensor.matmul(
                            sc[:, off:], lhsT=kro[:, t * P:(t + 1) * P],
                            rhs=qro[h][:, c * CH + off:(c + 1) * CH],
                            start=True, stop=True,
                        )
                        pb = pbp.tile([P, CH], bf16, tag="pb")
                        nc.scalar.activation(out=pb[:, off:], in_=sc[:, off:], func=Act.Exp)
                        if j >= 0:
                            # diagonal block: zero where sq < sk in the 128-col group
                            nc.vector.tensor_mul(pb[:, off:off + P], pb[:, off:off + P], tri_sb)
                        nc.tensor.matmul(csum[:, off:], lhsT=ones_bb, rhs=pb[:, off:],
                                         start=(t == 0), stop=(t == tmax - 1),
                                         skip_group_check=True)
                        nc.tensor.matmul(attps[:, off:], lhsT=v_sb[:, t, :], rhs=pb[:, off:],
                                         start=(t == 0), stop=(t == tmax - 1),
                                         skip_group_check=True)
                    # normalize: att = attps / colsum (reciprocal on DVE, ~2ulp)
                    rcp = wnorm.tile([P, CH], f32, tag="rn")
                    scr = wnorm.tile([P, CH], f32, tag="ln")
                    nc.vector.reciprocal_approx_accurate(out=rcp, in_=csum, scratch=scr)
                    nc.vector.tensor_mul(att_sb[h][:, cs], attps, rcp)

        # ---- Phase O (output projection) per chunk ----
        def emit_O(c):
                for st in range(4 * c, 4 * c + 4):
                    for mc in range(NCH):
                        ops = p_pa.tile([P, CH], f32, tag="pa")
                        for h in range(G):
                            nc.tensor.matmul(
                                ops, lhsT=att_sb[h][:, st * P:(st + 1) * P],
                                rhs=wo_sb[:, h, mc * CH:(mc + 1) * CH],
                                start=(h == 0), stop=(h == G - 1),
                            )
                        osb = osbp.tile([P, CH], bf16, tag="osb")
                        nc.vector.tensor_copy(osb, ops)
                        nc.sync.dma_start(
                            out=d_out[st * P:(st + 1) * P, mc * CH:(mc + 1) * CH], in_=osb)

        # A/B interleaved so B's ACT-heavy stretches overlap A's PE-heavy
        # matmuls; O phases emitted last = lowest priority, so the scheduler
        # uses their (dependency-free) matmuls to fill exp-latency bubbles.
        emit_A(0)
        emit_B(0)
        emit_A(1)
        emit_B(1)
        emit_A(2)
        emit_B(2)
        emit_A(3)
        emit_B(3)
        emit_O(0)
        emit_O(1)
        emit_O(2)
        emit_O(3)

    # Pin every activation to the one table set that contains all functions
    # we use (exp/ln/copy/square), so the ACT engine never swaps tables.
    # Indices must stay aligned with act_info.json, so other sets are kept
    # in place but emptied (the pass then can't pick them).
    from concourse import bacc as bacc_mod
    orig_tables = bacc_mod.get_activation_tables
    target = "natural_log_exp_and_others"

    def unified_tables(arch):
        t = orig_tables(arch)
        assert target in t
        return {k: (v if k == target else set()) for k, v in t.items()}

    bacc_mod.get_activation_tables = unified_tables
    try:
        nc.compile()
    finally:
        bacc_mod.get_activation_tables = orig_tables
    return nc


def _get_nc():
    if "nc" not in _CACHE:
        _CACHE["nc"] = _build_nc()
    return _CACHE["nc"]


def _rope_tables():
    inv_ts = THETA ** (-np.arange(HD // 2, dtype=np.float64) / (HD // 2))
    ang = np.arange(S, dtype=np.float64)[None, :] * inv_ts[:, None]  # [64, S]
    cos64 = np.cos(ang)
    sin64 = np.sin(ang)
    cos_t = np.concatenate([cos64, cos64], 0).astype(np.float32)
    # rotate-then-multiply signs: top rows get -sin, bottom +sin
    sin_t = np.concatenate([-sin64, sin64], 0).astype(np.float32)
    return cos_t, sin_t


def kernel(x, wq, wk, wv, wo, q_scale, k_scale):
    bf = ml_dtypes.bfloat16
    x = np.asarray(x, np.float32)
    wq = np.asarray(wq, np.float32)
    wk = np.asarray(wk, np.float32)
    wv = np.asarray(wv, np.float32)
    wo = np.asarray(wo, np.float32)
    q_scale = np.asarray(q_scale, np.float32)
    k_scale = np.asarray(k_scale, np.float32)

    from concourse.bass_utils import run_bass_kernel_spmd

    nc = _get_nc()
    cos_t, sin_t = _rope_tables()
    half = P // 2
    psw = np.zeros((P, P), np.float32)
    psw[np.arange(half) + half, np.arange(half)] = 1.0
    psw[np.arange(half), np.arange(half) + half] = 1.0
    tri = (np.arange(P)[None, :] >= np.arange(P)[:, None]).astype(np.float32)

    in_maps = []
    for core in range(8):
        b, g = divmod(core, 4)
        in_maps.append({
            "xt": np.ascontiguousarray(
                x[b].T.reshape(KT, P, NCH, CH).transpose(2, 1, 0, 3)).astype(bf),
            "wq4": np.ascontiguousarray(
                wq[:, 4 * g:4 * g + 4, :].reshape(KT, P, 2, 2, HD)
                .transpose(2, 1, 0, 3, 4)).astype(bf),
            "wk1": np.ascontiguousarray(
                wk[:, g, :].reshape(KT, P, HD).transpose(1, 0, 2)).astype(bf),
            "wv1": np.ascontiguousarray(
                wv[:, g, :].reshape(KT, P, HD).transpose(1, 0, 2)).astype(bf),
            "wo4": np.ascontiguousarray(np.transpose(wo[4 * g:4 * g + 4], (1, 0, 2))).astype(bf),
            "qsc": q_scale.reshape(HD, 1),
            "ksc": k_scale.reshape(HD, 1),
            "psw": psw.astype(bf),
            "cos_t": cos_t.astype(bf),
            "sin_t": sin_t.astype(bf),
            "tri": tri.astype(bf),
        })

    res = run_bass_kernel_spmd(nc, in_maps, list(range(8)), **_RUN_KWARGS)
    _CACHE["last_res"] = res
    out = np.zeros((B, S, DM), np.float32)
    for core in range(8):
        out[core // 4] += res.results[core]["o_part"].astype(np.float32)
    return out


# revision 42
# speedup vs baseline: 1.0202x; 1.0062x over previous
"""GQA attention layer (B=2,S=2048,D=2048,H=16,KV=4,HD=128) on 8 trn2 cores.

Sharding: core = (b, g) for b in {0,1} (batch), g in {0..3} (kv group).
Each core computes q-heads 4g..4g+3 + kv head g for batch b, producing a
partial o-projection [S, D] (bf16); the host sums the 4 partials per batch.

Per-core kernel: everything in transposed layout (head_dim on partitions),
bf16 matmuls with fp32 accumulation, softmax without max-subtraction
(logits bounded after RMSNorm), causal block skipping. Partition-dim
reductions (rms-norm sum-of-squares, softmax denominator) via ones-matmul
with M=128 so the result is already broadcast across partitions;
reciprocals/rsqrt computed on ScalarE as exp(-a*ln(x)).

v is projected in transposed layout (like k) so its weight loads hide
behind N=512 matmuls, then moved to natural [s, hd] layout with XBAR
DMA transposes (natural-layout v projection is load-dominated: N=128
matmuls cannot hide the 128-row weight loads). Input DMAs are split
across both HWDGE queues (sync: wk -> x0 quarters -> wq head-pair
halves -> x1,x3; scalar: scales -> wv -> rope tables -> tri -> x2 ->
wo), ordered so the tensor engine's first matmuls depend on minimal
bytes; all four x chunks have resident buffers so transfers never wait
on compute. cos/sin tables and the output partial are bf16 (error
budget allows it; halves that DMA traffic). O-projection phases are
emitted last = lowest priority, so the priority-heap scheduler uses
their dependency-free matmuls to fill exp-latency bubbles in the
attention tail; they share the projection PSUM pool (free by then)
while attention accumulation keeps its own, and output copies stay on
DVE because ACT saturates with exp during the B3/O overlap.
"""
import numpy as np
import ml_dtypes

B, S, DM = 2, 2048, 2048
H, KV, HD = 16, 4, 128
G = H // KV
THETA = 10000.0
EPS = 1e-6

P = 128         # partitions
CH = 512        # s-chunk (matmul N)
NCH = S // CH   # 4
KT = DM // P    # 16 contraction tiles
NST = S // P    # 16 s-tiles

_CACHE = {}
# extra kwargs for run_bass_kernel_spmd (test harness sets trace/tmpdir here)
_RUN_KWARGS = {}


def _build_nc():
    from concourse import bacc, mybir
    import concourse.tile as tile
    from contextlib import ExitStack

    f32 = mybir.dt.float32
    bf16 = mybir.dt.bfloat16
    Act = mybir.ActivationFunctionType

    nc = bacc.Bacc()
    d_xt = nc.declare_dram_parameter("xt", [NCH, P, KT, CH], bf16, isOutput=False)
    # wq stored head-pair-major: two contiguous 1MB blocks of [P, KT, 2, HD]
    # so q0/q1 projections can complete after the first block lands
    d_wq = nc.declare_dram_parameter("wq4", [2, P, KT, 2, HD], bf16, isOutput=False)
    d_wk = nc.declare_dram_parameter("wk1", [P, KT, HD], bf16, isOutput=False)
    d_wv = nc.declare_dram_parameter("wv1", [P, KT, HD], bf16, isOutput=False)
    d_wo = nc.declare_dram_parameter("wo4", [HD, G, DM], bf16, isOutput=False)
    d_qs = nc.declare_dram_parameter("qsc", [HD, 1], f32, isOutput=False)
    d_ks = nc.declare_dram_parameter("ksc", [HD, 1], f32, isOutput=False)
    d_psw = nc.declare_dram_parameter("psw", [P, P], bf16, isOutput=False)
    d_cos = nc.declare_dram_parameter("cos_t", [P, S], bf16, isOutput=False)
    d_sin = nc.declare_dram_parameter("sin_t", [P, S], bf16, isOutput=False)
    d_tri = nc.declare_dram_parameter("tri", [P, P], bf16, isOutput=False)
    d_out = nc.declare_dram_parameter("o_part", [S, DM], bf16, isOutput=True)

    with tile.TileContext(nc) as tc, ExitStack() as ctx:
        const = ctx.enter_context(tc.tile_pool(name="const", bufs=1))
        xin = ctx.enter_context(tc.tile_pool(name="xin", bufs=4))
        work = ctx.enter_context(tc.tile_pool(name="work", bufs=4))
        vtp = ctx.enter_context(tc.tile_pool(name="vtp", bufs=2))
        osbp = ctx.enter_context(tc.tile_pool(name="osbp", bufs=4))
        pbp = ctx.enter_context(tc.tile_pool(name="pbp", bufs=8))
        wnorm = ctx.enter_context(tc.tile_pool(name="wnorm", bufs=3))
        # PSUM: 8 banks. pa_ops shared by projections (A) + o-proj (O) so
        # attention accumulation (p_att) never blocks o-proj bubble-filling.
        p_pa = ctx.enter_context(tc.tile_pool(name="p_pa", bufs=2, space="PSUM"))
        p_sc = ctx.enter_context(tc.tile_pool(name="p_sc", bufs=3, space="PSUM"))
        p_red = ctx.enter_context(tc.tile_pool(name="p_red", bufs=1, space="PSUM"))
        p_att = ctx.enter_context(tc.tile_pool(name="p_att", bufs=2, space="PSUM"))

        # ---- persistent SBUF + input DMA schedule ----
        # sync ring (Q1): wk first (unblocks the first k-proj ~3us before the
        # scalar ring boots), then x0 quarters, wq halves, x1, x3.
        # scalar ring (Q10): wv + small tables, then x2 (rebalances the load
        # so Q10 doesn't idle while Q1 still streams x), then wo.
        wk_sb = const.tile([P, KT, HD], bf16, tag="wk_sb")
        nc.sync.dma_start(out=wk_sb, in_=d_wk[:])
        xt0 = xin.tile([P, KT, CH], bf16, tag="xt_c")
        for i in range(4):
            nc.sync.dma_start(out=xt0[:, 4 * i:4 * i + 4], in_=d_xt[0, :, 4 * i:4 * i + 4])
        wq_sb = [const.tile([P, KT, 2, HD], bf16, tag=f"wq_sb{i}", name=f"wq_sb{i}")
                 for i in range(2)]
        nc.sync.dma_start(out=wq_sb[0], in_=d_wq[0])
        nc.sync.dma_start(out=wq_sb[1], in_=d_wq[1])
        # all four x chunks are resident (xin bufs=4), so every x DMA streams
        # immediately instead of waiting for an earlier chunk to be consumed
        x_later = []
        for c in (1, 2, 3):
            xt_c = xin.tile([P, KT, CH], bf16, tag="xt_c", name=f"xt_c{c}")
            x_later.append(xt_c)
        nc.sync.dma_start(out=x_later[0], in_=d_xt[1])
        nc.sync.dma_start(out=x_later[2], in_=d_xt[3])

        qsc_sb = const.tile([HD, 1], f32, tag="qsc_sb")
        nc.scalar.dma_start(out=qsc_sb, in_=d_qs[:])
        ksc_sb = const.tile([HD, 1], f32, tag="ksc_sb")
        nc.scalar.dma_start(out=ksc_sb, in_=d_ks[:])
        wv_sb = const.tile([P, KT, HD], bf16, tag="wv_sb")
        nc.scalar.dma_start(out=wv_sb, in_=d_wv[:])
        psw_sb = const.tile([P, P], bf16, tag="psw_sb")
        nc.scalar.dma_start(out=psw_sb, in_=d_psw[:])
        cos_sb = const.tile([P, S], bf16, tag="cos_sb")
        nc.scalar.dma_start(out=cos_sb, in_=d_cos[:])
        sin_sb = const.tile([P, S], bf16, tag="sin_sb")
        nc.scalar.dma_start(out=sin_sb, in_=d_sin[:])
        tri_sb = const.tile([P, P], bf16, tag="tri_sb")
        nc.scalar.dma_start(out=tri_sb, in_=d_tri[:])
        nc.scalar.dma_start(out=x_later[1], in_=d_xt[2])
        wo_sb = const.tile([P, G, DM], bf16, tag="wo_sb")
        nc.scalar.dma_start(out=wo_sb, in_=d_wo[:])

        ones_bb = const.tile([P, P], bf16, tag="ones_bb")
        nc.vector.memset(ones_bb, 1.0)
        eps_q = const.tile([P, 1], f32, tag="eps_q")
        nc.vector.memset(eps_q, float(HD * EPS))
        eps_k = const.tile([P, 1], f32, tag="eps_k")
        nc.vector.memset(eps_k, float(EPS))

        # roped q heads / k / v / normalized att, persistent
        qro = [const.tile([P, S], bf16, tag=f"qro{h}", name=f"qro{h}") for h in range(G)]
        kro = const.tile([P, S], bf16, tag="kro")
        v_sb = const.tile([P, NST, HD], bf16, tag="v_sb")
        att_sb = [const.tile([P, S], bf16, tag=f"att{h}", name=f"att{h}") for h in range(G)]

        # ---- Phase A (projections+rmsnorm+rope) per chunk ----
        def emit_A(c):
                cs = slice(c * CH, (c + 1) * CH)
                xt_c = xt0 if c == 0 else x_later[c - 1]
                # order: k first (smallest weights, arrives earliest), then v
                # (transposed; weight loads hidden), then q heads.
                for h in (G, G + 1, 0, 1, 2, 3):
                    is_q = h < G
                    is_v = h == G + 1
                    ps_q = p_pa.tile([P, CH], f32, tag="pa")
                    for kt in range(KT):
                        if is_q:
                            lhs = wq_sb[h // 2][:, kt, h % 2, :]
                        elif is_v:
                            lhs = wv_sb[:, kt, :]
                        else:
                            lhs = wk_sb[:, kt, :]
                        nc.tensor.matmul(
                            ps_q, lhsT=lhs, rhs=xt_c[:, kt],
                            start=(kt == 0), stop=(kt == KT - 1),
                        )
                    if is_v:
                        # vT chunk -> bf16 SBUF -> natural [s, hd] via XBAR
                        vt_sb = vtp.tile([P, CH], bf16, tag="vt_sb")
                        nc.vector.tensor_copy(vt_sb, ps_q)
                        for st in range(4):
                            nc.sync.dma_start_transpose(
                                out=v_sb[:, 4 * c + st, :],
                                in_=vt_sb[:, st * P:(st + 1) * P])
                        continue
                    # rmsnorm: sumsq over hd via ones-matmul (M=128 -> broadcast rows)
                    qsq = wnorm.tile([P, CH], bf16, tag="qsq")
                    nc.scalar.activation(out=qsq, in_=ps_q, func=Act.Square)
                    ss = p_sc.tile([P, CH], f32, tag="sc")
                    nc.tensor.matmul(ss, lhsT=ones_bb, rhs=qsq, start=True, stop=True)
                    ln = wnorm.tile([P, CH], f32, tag="ln")
                    if is_q:
                        # rn = 1/sqrt(sumsq + HD*eps) == rmsnorm_scale * HD^-0.5
                        nc.scalar.activation(out=ln, in_=ss, func=Act.Ln,
                                             scale=1.0, bias=eps_q)
                    else:
                        nc.scalar.activation(out=ln, in_=ss, func=Act.Ln,
                                             scale=1.0 / HD, bias=eps_k)
                    rn = wnorm.tile([P, CH], f32, tag="rn")
                    nc.scalar.activation(out=rn, in_=ln, func=Act.Exp, scale=-0.5)
                    qs = work.tile([P, CH], bf16, tag="qs")
                    nc.vector.scalar_tensor_tensor(
                        out=qs, in0=ps_q, scalar=(qsc_sb if is_q else ksc_sb), in1=rn,
                        op0=mybir.AluOpType.mult, op1=mybir.AluOpType.mult)
                    # rope: out = qs*cos + swap(qs)*sin_signed (swap via PE permute)
                    rot = p_sc.tile([P, CH], f32, tag="sc")
                    nc.tensor.matmul(rot, lhsT=psw_sb, rhs=qs, start=True, stop=True)
                    t1 = work.tile([P, CH], f32, tag="t1")
                    nc.vector.tensor_mul(t1, qs, cos_sb[:, cs])
                    u = work.tile([P, CH], f32, tag="u")
                    nc.vector.tensor_mul(u, rot, sin_sb[:, cs])
                    dst = qro[h] if is_q else kro
                    nc.vector.tensor_add(dst[:, cs], t1, u)

        # ---- Phase B (attention) per chunk ----
        def emit_B(c):
                for h in range(G):
                    cs = slice(c * CH, (c + 1) * CH)
                    attps = p_att.tile([P, CH], f32, tag="att")
                    csum = p_red.tile([P, CH], f32, tag="cs")
                    tmax = 4 * c + 4
                    for t in range(tmax):
                        j = t - 4 * c
                        off = P * j if j > 0 else 0
                        sc = p_sc.tile([P, CH], f32, tag="sc")
                        nc.tensor.matmul(
                            sc[:, off:], lhsT=kro[:, t * P:(t + 1) * P],
                            rhs=qro[h][:, c * CH + off:(c + 1) * CH],
                            start=True, stop=True,
                        )
                        pb = pbp.tile([P, CH], bf16, tag="pb")
                        nc.scalar.activation(out=pb[:, off:], in_=sc[:, off:], func=Act.Exp)
                        if j >= 0:
                            # diagonal block: zero where sq < sk in the 128-col group
                            nc.vector.tensor_mul(pb[:, off:off + P], pb[:, off:off + P], tri_sb)
                        nc.tensor.matmul(csum[:, off:], lhsT=ones_bb, rhs=pb[:, off:],
                                         start=(t == 0), stop=(t == tmax - 1),
                                         skip_group_check=True)
                        nc.tensor.matmul(attps[:, off:], lhsT=v_sb[:, t, :], rhs=pb[:, off:],
                                         start=(t == 0), stop=(t == tmax - 1),
                                         skip_group_check=True)
                    # normalize: att = attps / colsum (reciprocal on DVE, ~2ulp)
                    rcp = wnorm.tile([P, CH], f32, tag="rn")
                    scr = wnorm.tile([P, CH], f32, tag="ln")
                    nc.vector.reciprocal_approx_accurate(out=rcp, in_=csum, scratch=scr)
                    nc.vector.tensor_mul(att_sb[h][:, cs], attps, rcp)

        # ---- Phase O (output projection) per chunk ----
        def emit_O(c):
                for st in range(4 * c, 4 * c + 4):
                    for mc in range(NCH):
                        ops = p_pa.tile([P, CH], f32, tag="pa")
                        for h in range(G):
                            nc.tensor.matmul(
                                ops, lhsT=att_sb[h][:, st * P:(st + 1) * P],
                                rhs=wo_sb[:, h, mc * CH:(mc + 1) * CH],
                                start=(h == 0), stop=(h == G - 1),
                            )
                        osb = osbp.tile([P, CH], bf16, tag="osb")
                        nc.vector.tensor_copy(osb, ops)
                        nc.sync.dma_start(
                            out=d_out[st * P:(st + 1) * P, mc * CH:(mc + 1) * CH], in_=osb)

        # A/B interleaved so B's ACT-heavy stretches overlap A's PE-heavy
        # matmuls; O phases emitted last = lowest priority, so the scheduler
        # uses their (dependency-free) matmuls to fill exp-latency bubbles.
        emit_A(0)
        emit_B(0)
        emit_A(1)
        emit_B(1)
        emit_A(2)
        emit_B(2)
        emit_A(3)
        emit_B(3)
        emit_O(0)
        emit_O(1)
        emit_O(2)
        emit_O(3)

    # Pin every activation to the one table set that contains all functions
    # we use (exp/ln/copy/square), so the ACT engine never swaps tables.
    # Indices must stay aligned with act_info.json, so other sets are kept
    # in place but emptied (the pass then can't pick them).
    from concourse import bacc as bacc_mod
    orig_tables = bacc_mod.get_activation_tables
    target = "natural_log_exp_and_others"

    def unified_tables(arch):
        t = orig_tables(arch)
        assert target in t
        return {k: (v if k == target else set()) for k, v in t.items()}

    bacc_mod.get_activation_tables = unified_tables
    try:
        nc.compile()
    finally:
        bacc_mod.get_activation_tables = orig_tables
    return nc


def _get_nc():
    if "nc" not in _CACHE:
        _CACHE["nc"] = _build_nc()
    return _CACHE["nc"]


def _rope_tables():
    inv_ts = THETA ** (-np.arange(HD // 2, dtype=np.float64) / (HD // 2))
    ang = np.arange(S, dtype=np.float64)[None, :] * inv_ts[:, None]  # [64, S]
    cos64 = np.cos(ang)
    sin64 = np.sin(ang)
    cos_t = np.concatenate([cos64, cos64], 0).astype(np.float32)
    # rotate-then-multiply signs: top rows get -sin, bottom +sin
    sin_t = np.concatenate([-sin64, sin64], 0).astype(np.float32)
    return cos_t, sin_t


def kernel(x, wq, wk, wv, wo, q_scale, k_scale):
    bf = ml_dtypes.bfloat16
    x = np.asarray(x, np.float32)
    wq = np.asarray(wq, np.float32)
    wk = np.asarray(wk, np.float32)
    wv = np.asarray(wv, np.float32)
    wo = np.asarray(wo, np.float32)
    q_scale = np.asarray(q_scale, np.float32)
    k_scale = np.asarray(k_scale, np.float32)

    from concourse.bass_utils import run_bass_kernel_spmd

    nc = _get_nc()
    cos_t, sin_t = _rope_tables()
    half = P // 2
    psw = np.zeros((P, P), np.float32)
    psw[np.arange(half) + half, np.arange(half)] = 1.0
    psw[np.arange(half), np.arange(half) + half] = 1.0
    tri = (np.arange(P)[None, :] >= np.arange(P)[:, None]).astype(np.float32)

    in_maps = []
    for core in range(8):
        b, g = divmod(core, 4)
        in_maps.append({
            "xt": np.ascontiguousarray(
                x[b].T.reshape(KT, P, NCH, CH).transpose(2, 1, 0, 3)).astype(bf),
            "wq4": np.ascontiguousarray(
                wq[:, 4 * g:4 * g + 4, :].reshape(KT, P, 2, 2, HD)
                .transpose(2, 1, 0, 3, 4)).astype(bf),
            "wk1": np.ascontiguousarray(
                wk[:, g, :].reshape(KT, P, HD).transpose(1, 0, 2)).astype(bf),
            "wv1": np.ascontiguousarray(
                wv[:, g, :].reshape(KT, P, HD).transpose(1, 0, 2)).astype(bf),
            "wo4": np.ascontiguousarray(np.transpose(wo[4 * g:4 * g + 4], (1, 0, 2))).astype(bf),
            "qsc": q_scale.reshape(HD, 1),
            "ksc": k_scale.reshape(HD, 1),
            "psw": psw.astype(bf),
            "cos_t": cos_t.astype(bf),
            "sin_t": sin_t.astype(bf),
            "tri": tri.astype(bf),
        })

    res = run_bass_kernel_spmd(nc, in_maps, list(range(8)), **_RUN_KWARGS)
    _CACHE["last_res"] = res
    out = np.zeros((B, S, DM), np.float32)
    for core in range(8):
        out[core // 4] += res.results[core]["o_part"].astype(np.float32)
    return out


# revision 50
# speedup vs baseline: 1.0241x; 1.0039x over previous
"""GQA attention layer (B=2,S=2048,D=2048,H=16,KV=4,HD=128) on 8 trn2 cores.

Sharding: core = (b, g) for b in {0,1} (batch), g in {0..3} (kv group).
Each core computes q-heads 4g..4g+3 + kv head g for batch b, producing a
partial o-projection [S, D] (bf16); the host sums the 4 partials per batch.

Per-core kernel: everything in transposed layout (head_dim on partitions),
bf16 matmuls with fp32 accumulation, softmax without max-subtraction
(logits bounded after RMSNorm), causal block skipping. Partition-dim
reductions (rms-norm sum-of-squares, softmax denominator) via ones-matmul
with M=128 so the result is already broadcast across partitions;
reciprocals/rsqrt computed on ScalarE as exp(-a*ln(x)).

v is projected in transposed layout (like k) so its weight loads hide
behind N=512 matmuls, then moved to natural [s, hd] layout with XBAR
DMA transposes (natural-layout v projection is load-dominated: N=128
matmuls cannot hide the 128-row weight loads). Input DMAs are split
across both HWDGE queues (sync: wk -> x0 quarters -> wq head-pair
halves -> x1,x3; scalar: scales -> wv -> rope tables -> tri -> x2 ->
wo), ordered so the tensor engine's first matmuls depend on minimal
bytes; all four x chunks have resident buffers so transfers never wait
on compute. cos/sin tables and the output partial are bf16 (error
budget allows it; halves that DMA traffic). O-projection phases are
emitted last = lowest priority, so the priority-heap scheduler uses
their dependency-free matmuls to fill exp-latency bubbles in the
attention tail; they share the projection PSUM pool (free by then)
while attention accumulation keeps its own, and output copies stay on
DVE because ACT saturates with exp during the B3/O overlap.
"""
import numpy as np
import ml_dtypes

B, S, DM = 2, 2048, 2048
H, KV, HD = 16, 4, 128
G = H // KV
THETA = 10000.0
EPS = 1e-6

P = 128         # partitions
CH = 512        # s-chunk (matmul N)
NCH = S // CH   # 4
KT = DM // P    # 16 contraction tiles
NST = S // P    # 16 s-tiles

_CACHE = {}
# extra kwargs for run_bass_kernel_spmd (test harness sets trace/tmpdir here)
_RUN_KWARGS = {}


def _build_nc():
    from concourse import bacc, mybir
    import concourse.tile as tile
    from contextlib import ExitStack

    f32 = mybir.dt.float32
    bf16 = mybir.dt.bfloat16
    Act = mybir.ActivationFunctionType

    nc = bacc.Bacc()
    d_xt = nc.declare_dram_parameter("xt", [NCH, P, KT, CH], bf16, isOutput=False)
    # wq stored head-pair-major: two contiguous 1MB blocks of [P, KT, 2, HD]
    # so q0/q1 projections can complete after the first block lands
    d_wq = nc.declare_dram_parameter("wq4", [2, P, KT, 2, HD], bf16, isOutput=False)
    d_wk = nc.declare_dram_parameter("wk1", [P, KT, HD], bf16, isOutput=False)
    d_wv = nc.declare_dram_parameter("wv1", [P, KT, HD], bf16, isOutput=False)
    d_wo = nc.declare_dram_parameter("wo4", [HD, G, DM], bf16, isOutput=False)
    d_qs = nc.declare_dram_parameter("qsc", [HD, 1], f32, isOutput=False)
    d_ks = nc.declare_dram_parameter("ksc", [HD, 1], f32, isOutput=False)
    d_psw = nc.declare_dram_parameter("psw", [P, P], bf16, isOutput=False)
    d_cos = nc.declare_dram_parameter("cos_t", [P, S], bf16, isOutput=False)
    d_sin = nc.declare_dram_parameter("sin_t", [P, S], bf16, isOutput=False)
    d_tri = nc.declare_dram_parameter("tri", [P, P], bf16, isOutput=False)
    d_out = nc.declare_dram_parameter("o_part", [S, DM], bf16, isOutput=True)

    with tile.TileContext(nc) as tc, ExitStack() as ctx:
        const = ctx.enter_context(tc.tile_pool(name="const", bufs=1))
        xin = ctx.enter_context(tc.tile_pool(name="xin", bufs=4))
        work = ctx.enter_context(tc.tile_pool(name="work", bufs=4))
        vtp = ctx.enter_context(tc.tile_pool(name="vtp", bufs=2))
        osbp = ctx.enter_context(tc.tile_pool(name="osbp", bufs=6))
        pbp = ctx.enter_context(tc.tile_pool(name="pbp", bufs=8))
        wnorm = ctx.enter_context(tc.tile_pool(name="wnorm", bufs=3))
        # PSUM: 8 banks. pa_ops shared by projections (A) + o-proj (O) so
        # attention accumulation (p_att) never blocks o-proj bubble-filling.
        p_pa = ctx.enter_context(tc.tile_pool(name="p_pa", bufs=2, space="PSUM"))
        p_sc = ctx.enter_context(tc.tile_pool(name="p_sc", bufs=3, space="PSUM"))
        p_red = ctx.enter_context(tc.tile_pool(name="p_red", bufs=1, space="PSUM"))
        p_att = ctx.enter_context(tc.tile_pool(name="p_att", bufs=2, space="PSUM"))

        # ---- persistent SBUF + input DMA schedule ----
        # sync ring (Q1): wk first (unblocks the first k-proj ~3us before the
        # scalar ring boots), then x0 quarters, wq halves, x1, x3.
        # scalar ring (Q10): wv + small tables, then x2 (rebalances the load
        # so Q10 doesn't idle while Q1 still streams x), then wo.
        wk_sb = const.tile([P, KT, HD], bf16, tag="wk_sb")
        nc.sync.dma_start(out=wk_sb, in_=d_wk[:])
        xt0 = xin.tile([P, KT, CH], bf16, tag="xt_c")
        for i in range(4):
            nc.sync.dma_start(out=xt0[:, 4 * i:4 * i + 4], in_=d_xt[0, :, 4 * i:4 * i + 4])
        wq_sb = [const.tile([P, KT, 2, HD], bf16, tag=f"wq_sb{i}", name=f"wq_sb{i}")
                 for i in range(2)]
        nc.sync.dma_start(out=wq_sb[0], in_=d_wq[0])
        nc.sync.dma_start(out=wq_sb[1], in_=d_wq[1])
        # all four x chunks are resident (xin bufs=4), so every x DMA streams
        # immediately instead of waiting for an earlier chunk to be consumed
        x_later = []
        for c in (1, 2, 3):
            xt_c = xin.tile([P, KT, CH], bf16, tag="xt_c", name=f"xt_c{c}")
            x_later.append(xt_c)
        nc.sync.dma_start(out=x_later[0], in_=d_xt[1])
        nc.sync.dma_start(out=x_later[2], in_=d_xt[3])

        qsc_sb = const.tile([HD, 1], f32, tag="qsc_sb")
        nc.scalar.dma_start(out=qsc_sb, in_=d_qs[:])
        ksc_sb = const.tile([HD, 1], f32, tag="ksc_sb")
        nc.scalar.dma_start(out=ksc_sb, in_=d_ks[:])
        wv_sb = const.tile([P, KT, HD], bf16, tag="wv_sb")
        nc.scalar.dma_start(out=wv_sb, in_=d_wv[:])
        psw_sb = const.tile([P, P], bf16, tag="psw_sb")
        nc.scalar.dma_start(out=psw_sb, in_=d_psw[:])
        cos_sb = const.tile([P, S], bf16, tag="cos_sb")
        nc.scalar.dma_start(out=cos_sb, in_=d_cos[:])
        sin_sb = const.tile([P, S], bf16, tag="sin_sb")
        nc.scalar.dma_start(out=sin_sb, in_=d_sin[:])
        tri_sb = const.tile([P, P], bf16, tag="tri_sb")
        nc.scalar.dma_start(out=tri_sb, in_=d_tri[:])
        nc.scalar.dma_start(out=x_later[1], in_=d_xt[2])
        wo_sb = const.tile([P, G, DM], bf16, tag="wo_sb")
        nc.scalar.dma_start(out=wo_sb, in_=d_wo[:])

        ones_bb = const.tile([P, P], bf16, tag="ones_bb")
        nc.vector.memset(ones_bb, 1.0)
        eps_q = const.tile([P, 1], f32, tag="eps_q")
        nc.vector.memset(eps_q, float(HD * EPS))
        eps_k = const.tile([P, 1], f32, tag="eps_k")
        nc.vector.memset(eps_k, float(EPS))

        # roped q heads / k / v / normalized att, persistent
        qro = [const.tile([P, S], bf16, tag=f"qro{h}", name=f"qro{h}") for h in range(G)]
        kro = const.tile([P, S], bf16, tag="kro")
        v_sb = const.tile([P, NST, HD], bf16, tag="v_sb")
        att_sb = [const.tile([P, S], bf16, tag=f"att{h}", name=f"att{h}") for h in range(G)]

        # ---- Phase A (projections+rmsnorm+rope) per chunk ----
        def emit_A(c):
                cs = slice(c * CH, (c + 1) * CH)
                xt_c = xt0 if c == 0 else x_later[c - 1]
                # order: k first (smallest weights, arrives earliest), then v
                # (transposed; weight loads hidden), then q heads.
                for h in (G, G + 1, 0, 1, 2, 3):
                    is_q = h < G
                    is_v = h == G + 1
                    ps_q = p_pa.tile([P, CH], f32, tag="pa")
                    for kt in range(KT):
                        if is_q:
                            lhs = wq_sb[h // 2][:, kt, h % 2, :]
                        elif is_v:
                            lhs = wv_sb[:, kt, :]
                        else:
                            lhs = wk_sb[:, kt, :]
                        nc.tensor.matmul(
                            ps_q, lhsT=lhs, rhs=xt_c[:, kt],
                            start=(kt == 0), stop=(kt == KT - 1),
                        )
                    if is_v:
                        # vT chunk -> bf16 SBUF -> natural [s, hd] via XBAR
                        vt_sb = vtp.tile([P, CH], bf16, tag="vt_sb")
                        nc.vector.tensor_copy(vt_sb, ps_q)
                        for st in range(4):
                            nc.sync.dma_start_transpose(
                                out=v_sb[:, 4 * c + st, :],
                                in_=vt_sb[:, st * P:(st + 1) * P])
                        continue
                    # rmsnorm: sumsq over hd via ones-matmul (M=128 -> broadcast rows)
                    qsq = wnorm.tile([P, CH], bf16, tag="qsq")
                    nc.scalar.activation(out=qsq, in_=ps_q, func=Act.Square)
                    ss = p_sc.tile([P, CH], f32, tag="sc")
                    nc.tensor.matmul(ss, lhsT=ones_bb, rhs=qsq, start=True, stop=True)
                    ln = wnorm.tile([P, CH], f32, tag="ln")
                    if is_q:
                        # rn = 1/sqrt(sumsq + HD*eps) == rmsnorm_scale * HD^-0.5
                        nc.scalar.activation(out=ln, in_=ss, func=Act.Ln,
                                             scale=1.0, bias=eps_q)
                    else:
                        nc.scalar.activation(out=ln, in_=ss, func=Act.Ln,
                                             scale=1.0 / HD, bias=eps_k)
                    rn = wnorm.tile([P, CH], f32, tag="rn")
                    nc.scalar.activation(out=rn, in_=ln, func=Act.Exp, scale=-0.5)
                    qs = work.tile([P, CH], bf16, tag="qs")
                    nc.vector.scalar_tensor_tensor(
                        out=qs, in0=ps_q, scalar=(qsc_sb if is_q else ksc_sb), in1=rn,
                        op0=mybir.AluOpType.mult, op1=mybir.AluOpType.mult)
                    # rope: out = qs*cos + swap(qs)*sin_signed (swap via PE permute)
                    rot = p_sc.tile([P, CH], f32, tag="sc")
                    nc.tensor.matmul(rot, lhsT=psw_sb, rhs=qs, start=True, stop=True)
                    t1 = work.tile([P, CH], f32, tag="t1")
                    nc.vector.tensor_mul(t1, qs, cos_sb[:, cs])
                    u = work.tile([P, CH], f32, tag="u")
                    nc.vector.tensor_mul(u, rot, sin_sb[:, cs])
                    dst = qro[h] if is_q else kro
                    nc.vector.tensor_add(dst[:, cs], t1, u)

        # ---- Phase B (attention) per chunk ----
        def emit_B(c):
                for h in range(G):
                    cs = slice(c * CH, (c + 1) * CH)
                    attps = p_att.tile([P, CH], f32, tag="att")
                    csum = p_red.tile([P, CH], f32, tag="cs")
                    tmax = 4 * c + 4
                    for t in range(tmax):
                        j = t - 4 * c
                        off = P * j if j > 0 else 0
                        sc = p_sc.tile([P, CH], f32, tag="sc")
                        nc.tensor.matmul(
                            sc[:, off:], lhsT=kro[:, t * P:(t + 1) * P],
                            rhs=qro[h][:, c * CH + off:(c + 1) * CH],
                            start=True, stop=True,
                        )
                        pb = pbp.tile([P, CH], bf16, tag="pb")
                        nc.scalar.activation(out=pb[:, off:], in_=sc[:, off:], func=Act.Exp)
                        if j >= 0:
                            # diagonal block: zero where sq < sk in the 128-col group
                            nc.vector.tensor_mul(pb[:, off:off + P], pb[:, off:off + P], tri_sb)
                        nc.tensor.matmul(csum[:, off:], lhsT=ones_bb, rhs=pb[:, off:],
                                         start=(t == 0), stop=(t == tmax - 1),
                                         skip_group_check=True)
                        nc.tensor.matmul(attps[:, off:], lhsT=v_sb[:, t, :], rhs=pb[:, off:],
                                         start=(t == 0), stop=(t == tmax - 1),
                                         skip_group_check=True)
                    # normalize: att = attps / colsum (reciprocal on DVE, ~2ulp)
                    rcp = wnorm.tile([P, CH], f32, tag="rn")
                    scr = wnorm.tile([P, CH], f32, tag="ln")
                    nc.vector.reciprocal_approx_accurate(out=rcp, in_=csum, scratch=scr)
                    nc.vector.tensor_mul(att_sb[h][:, cs], attps, rcp)

        # ---- Phase O (output projection) per chunk ----
        def emit_O(c):
                for st in range(4 * c, 4 * c + 4):
                    for mc in range(NCH):
                        ops = p_pa.tile([P, CH], f32, tag="pa")
                        for h in range(G):
                            nc.tensor.matmul(
                                ops, lhsT=att_sb[h][:, st * P:(st + 1) * P],
                                rhs=wo_sb[:, h, mc * CH:(mc + 1) * CH],
                                start=(h == 0), stop=(h == G - 1),
                            )
                        osb = osbp.tile([P, CH], bf16, tag="osb")
                        nc.vector.tensor_copy(osb, ops)
                        eng = nc.sync if mc % 2 == 0 else nc.scalar
                        eng.dma_start(
                            out=d_out[st * P:(st + 1) * P, mc * CH:(mc + 1) * CH], in_=osb)

        # A/B interleaved so B's ACT-heavy stretches overlap A's PE-heavy
        # matmuls; O phases emitted last = lowest priority, so the scheduler
        # uses their (dependency-free) matmuls to fill exp-latency bubbles.
        emit_A(0)
        emit_B(0)
        emit_A(1)
        emit_B(1)
        emit_A(2)
        emit_B(2)
        emit_A(3)
        emit_B(3)
        emit_O(0)
        emit_O(1)
        emit_O(2)
        emit_O(3)

    # Pin every activation to the one table set that contains all functions
    # we use (exp/ln/copy/square), so the ACT engine never swaps tables.
    # Indices must stay aligned with act_info.json, so other sets are kept
    # in place but emptied (the pass then can't pick them).
    from concourse import bacc as bacc_mod
    orig_tables = bacc_mod.get_activation_tables
    target = "natural_log_exp_and_others"

    def unified_tables(arch):
        t = orig_tables(arch)
        assert target in t
        return {k: (v if k == target else set()) for k, v in t.items()}

    bacc_mod.get_activation_tables = unified_tables
    try:
        nc.compile()
    finally:
        bacc_mod.get_activation_tables = orig_tables
    return nc


def _get_nc():
    if "nc" not in _CACHE:
        _CACHE["nc"] = _build_nc()
    return _CACHE["nc"]


def _rope_tables():
    inv_ts = THETA ** (-np.arange(HD // 2, dtype=np.float64) / (HD // 2))
    ang = np.arange(S, dtype=np.float64)[None, :] * inv_ts[:, None]  # [64, S]
    cos64 = np.cos(ang)
    sin64 = np.sin(ang)
    cos_t = np.concatenate([cos64, cos64], 0).astype(np.float32)
    # rotate-then-multiply signs: top rows get -sin, bottom +sin
    sin_t = np.concatenate([-sin64, sin64], 0).astype(np.float32)
    return cos_t, sin_t


def kernel(x, wq, wk, wv, wo, q_scale, k_scale):
    bf = ml_dtypes.bfloat16
    x = np.asarray(x, np.float32)
    wq = np.asarray(wq, np.float32)
    wk = np.asarray(wk, np.float32)
    wv = np.asarray(wv, np.float32)
    wo = np.asarray(wo, np.float32)
    q_scale = np.asarray(q_scale, np.float32)
    k_scale = np.asarray(k_scale, np.float32)

    from concourse.bass_utils import run_bass_kernel_spmd

    nc = _get_nc()
    cos_t, sin_t = _rope_tables()
    half = P // 2
    psw = np.zeros((P, P), np.float32)
    psw[np.arange(half) + half, np.arange(half)] = 1.0
    psw[np.arange(half), np.arange(half) + half] = 1.0
    tri = (np.arange(P)[None, :] >= np.arange(P)[:, None]).astype(np.float32)

    in_maps = []
    for core in range(8):
        b, g = divmod(core, 4)
        in_maps.append({
            "xt": np.ascontiguousarray(
                x[b].T.reshape(KT, P, NCH, CH).transpose(2, 1, 0, 3)).astype(bf),
            "wq4": np.ascontiguousarray(
                wq[:, 4 * g:4 * g + 4, :].reshape(KT, P, 2, 2, HD)
                .transpose(2, 1, 0, 3, 4)).astype(bf),
            "wk1": np.ascontiguousarray(
                wk[:, g, :].reshape(KT, P, HD).transpose(1, 0, 2)).astype(bf),
            "wv1": np.ascontiguousarray(
                wv[:, g, :].reshape(KT, P, HD).transpose(1, 0, 2)).astype(bf),
            "wo4": np.ascontiguousarray(np.transpose(wo[4 * g:4 * g + 4], (1, 0, 2))).astype(bf),
            "qsc": q_scale.reshape(HD, 1),
            "ksc": k_scale.reshape(HD, 1),
            "psw": psw.astype(bf),
            "cos_t": cos_t.astype(bf),
            "sin_t": sin_t.astype(bf),
            "tri": tri.astype(bf),
        })

    res = run_bass_kernel_spmd(nc, in_maps, list(range(8)), **_RUN_KWARGS)
    _CACHE["last_res"] = res
    out = np.zeros((B, S, DM), np.float32)
    for core in range(8):
        out[core // 4] += res.results[core]["o_part"].astype(np.float32)
    return out


# revision 53
# speedup vs baseline: 1.0262x; 1.0020x over previous
"""GQA attention layer (B=2,S=2048,D=2048,H=16,KV=4,HD=128) on 8 trn2 cores.

Sharding: core = (b, g) for b in {0,1} (batch), g in {0..3} (kv group).
Each core computes q-heads 4g..4g+3 + kv head g for batch b, producing a
partial o-projection [S, D] (bf16); the host sums the 4 partials per batch.

Per-core kernel: everything in transposed layout (head_dim on partitions),
bf16 matmuls with fp32 accumulation, softmax without max-subtraction
(logits bounded after RMSNorm), causal block skipping. Partition-dim
reductions (rms-norm sum-of-squares, softmax denominator) via ones-matmul
with M=128 so the result is already broadcast across partitions;
reciprocals/rsqrt computed on ScalarE as exp(-a*ln(x)).

v is projected in transposed layout (like k) so its weight loads hide
behind N=512 matmuls, then moved to natural [s, hd] layout with XBAR
DMA transposes (natural-layout v projection is load-dominated: N=128
matmuls cannot hide the 128-row weight loads). Input DMAs are split
across both HWDGE queues (sync: wk -> x0 quarters -> wq head-pair
halves -> x1,x3; scalar: scales -> wv -> rope tables -> tri -> x2 ->
wo), ordered so the tensor engine's first matmuls depend on minimal
bytes; all four x chunks have resident buffers so transfers never wait
on compute. cos/sin tables and the output partial are bf16 (error
budget allows it; halves that DMA traffic). O-projection phases are
emitted last = lowest priority, so the priority-heap scheduler uses
their dependency-free matmuls to fill exp-latency bubbles in the
attention tail; they share the projection PSUM pool (free by then)
while attention accumulation keeps its own, and output copies stay on
DVE because ACT saturates with exp during the B3/O overlap.
"""
import numpy as np
import ml_dtypes

B, S, DM = 2, 2048, 2048
H, KV, HD = 16, 4, 128
G = H // KV
THETA = 10000.0
EPS = 1e-6

P = 128         # partitions
CH = 512        # s-chunk (matmul N)
NCH = S // CH   # 4
KT = DM // P    # 16 contraction tiles
NST = S // P    # 16 s-tiles

_CACHE = {}
# extra kwargs for run_bass_kernel_spmd (test harness sets trace/tmpdir here)
_RUN_KWARGS = {}


def _build_nc():
    from concourse import bacc, mybir
    import concourse.tile as tile
    from contextlib import ExitStack

    f32 = mybir.dt.float32
    bf16 = mybir.dt.bfloat16
    Act = mybir.ActivationFunctionType

    nc = bacc.Bacc()
    d_xt = nc.declare_dram_parameter("xt", [NCH, P, KT, CH], bf16, isOutput=False)
    # wq stored head-major: four contiguous 512KB blocks of [P, KT, HD]
    # so each q projection can complete as soon as its own block lands
    d_wq = nc.declare_dram_parameter("wq4", [G, P, KT, HD], bf16, isOutput=False)
    d_wk = nc.declare_dram_parameter("wk1", [P, KT, HD], bf16, isOutput=False)
    d_wv = nc.declare_dram_parameter("wv1", [P, KT, HD], bf16, isOutput=False)
    d_wo = nc.declare_dram_parameter("wo4", [HD, G, DM], bf16, isOutput=False)
    d_qs = nc.declare_dram_parameter("qsc", [HD, 1], f32, isOutput=False)
    d_ks = nc.declare_dram_parameter("ksc", [HD, 1], f32, isOutput=False)
    d_psw = nc.declare_dram_parameter("psw", [P, P], bf16, isOutput=False)
    d_cos = nc.declare_dram_parameter("cos_t", [P, S], bf16, isOutput=False)
    d_sin = nc.declare_dram_parameter("sin_t", [P, S], bf16, isOutput=False)
    d_tri = nc.declare_dram_parameter("tri", [P, P], bf16, isOutput=False)
    d_out = nc.declare_dram_parameter("o_part", [S, DM], bf16, isOutput=True)

    with tile.TileContext(nc) as tc, ExitStack() as ctx:
        const = ctx.enter_context(tc.tile_pool(name="const", bufs=1))
        xin = ctx.enter_context(tc.tile_pool(name="xin", bufs=4))
        work = ctx.enter_context(tc.tile_pool(name="work", bufs=4))
        vtp = ctx.enter_context(tc.tile_pool(name="vtp", bufs=2))
        osbp = ctx.enter_context(tc.tile_pool(name="osbp", bufs=6))
        pbp = ctx.enter_context(tc.tile_pool(name="pbp", bufs=8))
        wnorm = ctx.enter_context(tc.tile_pool(name="wnorm", bufs=3))
        # PSUM: 8 banks. pa_ops shared by projections (A) + o-proj (O) so
        # attention accumulation (p_att) never blocks o-proj bubble-filling.
        p_pa = ctx.enter_context(tc.tile_pool(name="p_pa", bufs=2, space="PSUM"))
        p_sc = ctx.enter_context(tc.tile_pool(name="p_sc", bufs=3, space="PSUM"))
        p_red = ctx.enter_context(tc.tile_pool(name="p_red", bufs=1, space="PSUM"))
        p_att = ctx.enter_context(tc.tile_pool(name="p_att", bufs=2, space="PSUM"))

        # ---- persistent SBUF + input DMA schedule ----
        # sync ring (Q1): wk first (unblocks the first k-proj ~3us before the
        # scalar ring boots), then x0 quarters, wq halves, x1, x3.
        # scalar ring (Q10): wv + small tables, then x2 (rebalances the load
        # so Q10 doesn't idle while Q1 still streams x), then wo.
        wk_sb = const.tile([P, KT, HD], bf16, tag="wk_sb")
        nc.sync.dma_start(out=wk_sb, in_=d_wk[:])
        xt0 = xin.tile([P, KT, CH], bf16, tag="xt_c")
        for i in range(8):
            nc.sync.dma_start(out=xt0[:, 2 * i:2 * i + 2], in_=d_xt[0, :, 2 * i:2 * i + 2])
        wq_sb = [const.tile([P, KT, HD], bf16, tag=f"wq_sb{i}", name=f"wq_sb{i}")
                 for i in range(G)]
        # all four x chunks are resident (xin bufs=4), so every x DMA streams
        # immediately instead of waiting for an earlier chunk to be consumed
        x_later = []
        for c in (1, 2, 3):
            xt_c = xin.tile([P, KT, CH], bf16, tag="xt_c", name=f"xt_c{c}")
            x_later.append(xt_c)
        for i in range(G):
            nc.sync.dma_start(out=wq_sb[i], in_=d_wq[i])
        nc.sync.dma_start(out=x_later[0], in_=d_xt[1])
        nc.sync.dma_start(out=x_later[2], in_=d_xt[3])

        qsc_sb = const.tile([HD, 1], f32, tag="qsc_sb")
        nc.scalar.dma_start(out=qsc_sb, in_=d_qs[:])
        ksc_sb = const.tile([HD, 1], f32, tag="ksc_sb")
        nc.scalar.dma_start(out=ksc_sb, in_=d_ks[:])
        wv_sb = const.tile([P, KT, HD], bf16, tag="wv_sb")
        nc.scalar.dma_start(out=wv_sb, in_=d_wv[:])
        psw_sb = const.tile([P, P], bf16, tag="psw_sb")
        nc.scalar.dma_start(out=psw_sb, in_=d_psw[:])
        cos_sb = const.tile([P, S], bf16, tag="cos_sb")
        nc.scalar.dma_start(out=cos_sb, in_=d_cos[:])
        sin_sb = const.tile([P, S], bf16, tag="sin_sb")
        nc.scalar.dma_start(out=sin_sb, in_=d_sin[:])
        tri_sb = const.tile([P, P], bf16, tag="tri_sb")
        nc.scalar.dma_start(out=tri_sb, in_=d_tri[:])
        nc.scalar.dma_start(out=x_later[1], in_=d_xt[2])
        wo_sb = const.tile([P, G, DM], bf16, tag="wo_sb")
        nc.scalar.dma_start(out=wo_sb, in_=d_wo[:])

        ones_bb = const.tile([P, P], bf16, tag="ones_bb")
        nc.vector.memset(ones_bb, 1.0)
        eps_q = const.tile([P, 1], f32, tag="eps_q")
        nc.vector.memset(eps_q, float(HD * EPS))
        eps_k = const.tile([P, 1], f32, tag="eps_k")
        nc.vector.memset(eps_k, float(EPS))

        # roped q heads / k / v / normalized att, persistent
        qro = [const.tile([P, S], bf16, tag=f"qro{h}", name=f"qro{h}") for h in range(G)]
        kro = const.tile([P, S], bf16, tag="kro")
        v_sb = const.tile([P, NST, HD], bf16, tag="v_sb")
        att_sb = [const.tile([P, S], bf16, tag=f"att{h}", name=f"att{h}") for h in range(G)]

        # ---- Phase A (projections+rmsnorm+rope) per chunk ----
        def emit_A(c):
                cs = slice(c * CH, (c + 1) * CH)
                xt_c = xt0 if c == 0 else x_later[c - 1]
                # order: k first (smallest weights, arrives earliest), then v
                # (transposed; weight loads hidden), then q heads.
                for h in (G, G + 1, 0, 1, 2, 3):
                    is_q = h < G
                    is_v = h == G + 1
                    ps_q = p_pa.tile([P, CH], f32, tag="pa")
                    for kt in range(KT):
                        if is_q:
                            lhs = wq_sb[h][:, kt, :]
                        elif is_v:
                            lhs = wv_sb[:, kt, :]
                        else:
                            lhs = wk_sb[:, kt, :]
                        nc.tensor.matmul(
                            ps_q, lhsT=lhs, rhs=xt_c[:, kt],
                            start=(kt == 0), stop=(kt == KT - 1),
                        )
                    if is_v:
                        # vT chunk -> bf16 SBUF -> natural [s, hd] via XBAR
                        vt_sb = vtp.tile([P, CH], bf16, tag="vt_sb")
                        nc.vector.tensor_copy(vt_sb, ps_q)
                        for st in range(4):
                            nc.sync.dma_start_transpose(
                                out=v_sb[:, 4 * c + st, :],
                                in_=vt_sb[:, st * P:(st + 1) * P])
                        continue
                    # rmsnorm: sumsq over hd via ones-matmul (M=128 -> broadcast rows)
                    qsq = wnorm.tile([P, CH], bf16, tag="qsq")
                    nc.scalar.activation(out=qsq, in_=ps_q, func=Act.Square)
                    ss = p_sc.tile([P, CH], f32, tag="sc")
                    nc.tensor.matmul(ss, lhsT=ones_bb, rhs=qsq, start=True, stop=True)
                    ln = wnorm.tile([P, CH], f32, tag="ln")
                    if is_q:
                        # rn = 1/sqrt(sumsq + HD*eps) == rmsnorm_scale * HD^-0.5
                        nc.scalar.activation(out=ln, in_=ss, func=Act.Ln,
                                             scale=1.0, bias=eps_q)
                    else:
                        nc.scalar.activation(out=ln, in_=ss, func=Act.Ln,
                                             scale=1.0 / HD, bias=eps_k)
                    rn = wnorm.tile([P, CH], f32, tag="rn")
                    nc.scalar.activation(out=rn, in_=ln, func=Act.Exp, scale=-0.5)
                    qs = work.tile([P, CH], bf16, tag="qs")
                    nc.vector.scalar_tensor_tensor(
                        out=qs, in0=ps_q, scalar=(qsc_sb if is_q else ksc_sb), in1=rn,
                        op0=mybir.AluOpType.mult, op1=mybir.AluOpType.mult)
                    # rope: out = qs*cos + swap(qs)*sin_signed (swap via PE permute)
                    rot = p_sc.tile([P, CH], f32, tag="sc")
                    nc.tensor.matmul(rot, lhsT=psw_sb, rhs=qs, start=True, stop=True)
                    t1 = work.tile([P, CH], f32, tag="t1")
                    nc.vector.tensor_mul(t1, qs, cos_sb[:, cs])
                    u = work.tile([P, CH], f32, tag="u")
                    nc.vector.tensor_mul(u, rot, sin_sb[:, cs])
                    dst = qro[h] if is_q else kro
                    nc.vector.tensor_add(dst[:, cs], t1, u)

        # ---- Phase B (attention) per chunk ----
        def emit_B(c):
                for h in range(G):
                    cs = slice(c * CH, (c + 1) * CH)
                    attps = p_att.tile([P, CH], f32, tag="att")
                    csum = p_red.tile([P, CH], f32, tag="cs")
                    tmax = 4 * c + 4
                    for t in range(tmax):
                        j = t - 4 * c
                        off = P * j if j > 0 else 0
                        sc = p_sc.tile([P, CH], f32, tag="sc")
                        nc.tensor.matmul(
                            sc[:, off:], lhsT=kro[:, t * P:(t + 1) * P],
                            rhs=qro[h][:, c * CH + off:(c + 1) * CH],
                            start=True, stop=True,
                        )
                        pb = pbp.tile([P, CH], bf16, tag="pb")
                        nc.scalar.activation(out=pb[:, off:], in_=sc[:, off:], func=Act.Exp)
                        if j >= 0:
                            # diagonal block: zero where sq < sk in the 128-col group
                            nc.vector.tensor_mul(pb[:, off:off + P], pb[:, off:off + P], tri_sb)
                        nc.tensor.matmul(csum[:, off:], lhsT=ones_bb, rhs=pb[:, off:],
                                         start=(t == 0), stop=(t == tmax - 1),
                                         skip_group_check=True)
                        nc.tensor.matmul(attps[:, off:], lhsT=v_sb[:, t, :], rhs=pb[:, off:],
                                         start=(t == 0), stop=(t == tmax - 1),
                                         skip_group_check=True)
                    # normalize: att = attps / colsum (reciprocal on DVE, ~2ulp)
                    rcp = wnorm.tile([P, CH], f32, tag="rn")
                    scr = wnorm.tile([P, CH], f32, tag="ln")
                    nc.vector.reciprocal_approx_accurate(out=rcp, in_=csum, scratch=scr)
                    nc.vector.tensor_mul(att_sb[h][:, cs], attps, rcp)

        # ---- Phase O (output projection) per chunk ----
        def emit_O(c):
                for st in range(4 * c, 4 * c + 4):
                    for mc in range(NCH):
                        ops = p_pa.tile([P, CH], f32, tag="pa")
                        for h in range(G):
                            nc.tensor.matmul(
                                ops, lhsT=att_sb[h][:, st * P:(st + 1) * P],
                                rhs=wo_sb[:, h, mc * CH:(mc + 1) * CH],
                                start=(h == 0), stop=(h == G - 1),
                            )
                        osb = osbp.tile([P, CH], bf16, tag="osb")
                        nc.vector.tensor_copy(osb, ops)
                        eng = nc.sync if mc % 2 == 0 else nc.scalar
                        eng.dma_start(
                            out=d_out[st * P:(st + 1) * P, mc * CH:(mc + 1) * CH], in_=osb)

        # A/B interleaved so B's ACT-heavy stretches overlap A's PE-heavy
        # matmuls; O phases emitted last = lowest priority, so the scheduler
        # uses their (dependency-free) matmuls to fill exp-latency bubbles.
        emit_A(0)
        emit_B(0)
        emit_A(1)
        emit_B(1)
        emit_A(2)
        emit_B(2)
        emit_A(3)
        emit_B(3)
        emit_O(0)
        emit_O(1)
        emit_O(2)
        emit_O(3)

    # Pin every activation to the one table set that contains all functions
    # we use (exp/ln/copy/square), so the ACT engine never swaps tables.
    # Indices must stay aligned with act_info.json, so other sets are kept
    # in place but emptied (the pass then can't pick them).
    from concourse import bacc as bacc_mod
    orig_tables = bacc_mod.get_activation_tables
    target = "natural_log_exp_and_others"

    def unified_tables(arch):
        t = orig_tables(arch)
        assert target in t
        return {k: (v if k == target else set()) for k, v in t.items()}

    bacc_mod.get_activation_tables = unified_tables
    try:
        nc.compile()
    finally:
        bacc_mod.get_activation_tables = orig_tables
    return nc


def _get_nc():
    if "nc" not in _CACHE:
        _CACHE["nc"] = _build_nc()
    return _CACHE["nc"]


def _rope_tables():
    inv_ts = THETA ** (-np.arange(HD // 2, dtype=np.float64) / (HD // 2))
    ang = np.arange(S, dtype=np.float64)[None, :] * inv_ts[:, None]  # [64, S]
    cos64 = np.cos(ang)
    sin64 = np.sin(ang)
    cos_t = np.concatenate([cos64, cos64], 0).astype(np.float32)
    # rotate-then-multiply signs: top rows get -sin, bottom +sin
    sin_t = np.concatenate([-sin64, sin64], 0).astype(np.float32)
    return cos_t, sin_t


def kernel(x, wq, wk, wv, wo, q_scale, k_scale):
    bf = ml_dtypes.bfloat16
    x = np.asarray(x, np.float32)
    wq = np.asarray(wq, np.float32)
    wk = np.asarray(wk, np.float32)
    wv = np.asarray(wv, np.float32)
    wo = np.asarray(wo, np.float32)
    q_scale = np.asarray(q_scale, np.float32)
    k_scale = np.asarray(k_scale, np.float32)

    from concourse.bass_utils import run_bass_kernel_spmd

    nc = _get_nc()
    cos_t, sin_t = _rope_tables()
    half = P // 2
    psw = np.zeros((P, P), np.float32)
    psw[np.arange(half) + half, np.arange(half)] = 1.0
    psw[np.arange(half), np.arange(half) + half] = 1.0
    tri = (np.arange(P)[None, :] >= np.arange(P)[:, None]).astype(np.float32)

    in_maps = []
    for core in range(8):
        b, g = divmod(core, 4)
        in_maps.append({
            "xt": np.ascontiguousarray(
                x[b].T.reshape(KT, P, NCH, CH).transpose(2, 1, 0, 3)).astype(bf),
            "wq4": np.ascontiguousarray(
                wq[:, 4 * g:4 * g + 4, :].reshape(KT, P, G, HD)
                .transpose(2, 1, 0, 3)).astype(bf),
            "wk1": np.ascontiguousarray(
                wk[:, g, :].reshape(KT, P, HD).transpose(1, 0, 2)).astype(bf),
            "wv1": np.ascontiguousarray(
                wv[:, g, :].reshape(KT, P, HD).transpose(1, 0, 2)).astype(bf),
            "wo4": np.ascontiguousarray(np.transpose(wo[4 * g:4 * g + 4], (1, 0, 2))).astype(bf),
            "qsc": q_scale.reshape(HD, 1),
            "ksc": k_scale.reshape(HD, 1),
            "psw": psw.astype(bf),
            "cos_t": cos_t.astype(bf),
            "sin_t": sin_t.astype(bf),
            "tri": tri.astype(bf),
        })

    res = run_bass_kernel_spmd(nc, in_maps, list(range(8)), **_RUN_KWARGS)
    _CACHE["last_res"] = res
    out = np.zeros((B, S, DM), np.float32)
    for core in range(8):
        out[core // 4] += res.results[core]["o_part"].astype(np.float32)
    return out
